# revision 33
# baseline (speedup 1.0000x reference)
"""Trainium2 Bass kernel for nn_ActorCritic loss_fn.

Strategy
--------
The reference computes a reverse discounted-return scan over time (T=8192),
normalizes the returns by masked global mean/std, and reduces to two scalar
losses. Both losses are polynomial in 10 masked global sums involving the raw
(unnormalized) returns R:

    N     = sum(m)          S1   = sum(m*R)       S2   = sum(m*R^2)
    SV    = sum(m*V)        SRV  = sum(m*R*V)     SV2  = sum(m*V^2)
    SLP   = sum(m*lp)       SLPR = sum(m*lp*R)    SLPV = sum(m*lp*V)
    SE    = sum(m*e)

so the device kernel is ONE streaming pass: compute R on the fly, form masked
products, reduce. Final scalar math happens on host in float64.

Sharding: batch dim split 8 ways -> (8192, 512) per core, streamed once.

Per-core pipeline (time tiled into 64 chunks of 128 rows = partition dim):
  SP  : streams inputs per 512-row super-chunk, double-buffered.
        rewards as f32(r); V/lp/e/mask pre-cast to bf16 on host.
  PE  : reverse scan per chunk as fp32r matmul with a lower-triangular
        gamma-power matrix; cross-chunk carry as a second fp32r matmul with a
        row-selector matrix (Sel[q,p] = gamma^(P-p) iff q==0) against the
        previous chunk's f32 R tile, accumulated into the same PSUM bank.
        Also: 8 of the 10 stat reductions as bf16 ones-column matmuls
        accumulating into a shared PSUM stats bank across all 64 chunks.
  ACT : copies R PSUM->SBUF twice (f32r for the carry chain, bf16 for the
        products) and does 2 stat reductions via activation accum_out.
  DVE : 7 bf16 elementwise products (2x perf mode).
  GPS : 2 bf16 elementwise products.

Raw Bass with manual semaphores and standalone wait_ge instructions: this
container's walrus build allows only ONE sync wait per instruction, which
rules out the Tile layer's fused-wait style.

Numerics: products in bf16 with f32 accumulation; scan in fp32r (~2e-5 rms).
Expected end-to-end relative error ~1e-5 vs the f32 reference.
"""

import numpy as np
from contextlib import ExitStack

GAMMA = 0.99
ALPHA = 0.01
EPS = 1e-8

T = 8192
B = 4096
NCORES = 8
BL = B // NCORES        # 512 batch columns per core
P = 128                 # time rows per scan chunk (SBUF partition dim)
KPC = 4                 # chunks per DMA super-chunk (512 rows)
NSUPER = T // (P * KPC)  # 16
NCHUNK = T // P          # 64

# PE-reduced stats (rows of the PSUM stats bank, via ones-column matmuls)
PE_STATS = ("N", "S1", "SV", "SLP", "S2", "SRV", "SLPR", "SV2")
NPE = len(PE_STATS)
# ACT-reduced stats (activation accum_out, per-chunk columns in `acc`)
ACT_STATS = ("SE", "SLPV")
NACT = len(ACT_STATS)

_cache = {}


def _build_program():
    import concourse.bass as bass
    import concourse.mybir as mybir

    dt = mybir.dt
    f32 = dt.float32
    f32r = dt.float32r
    bf16 = dt.bfloat16
    mult = mybir.AluOpType.mult
    Copy = mybir.ActivationFunctionType.Copy

    nc = bass.Bass()
    r_d = nc.dram_tensor("rewards", [T, BL], f32r, kind="ExternalInput")
    v_d = nc.dram_tensor("value_estimates", [T, BL], bf16, kind="ExternalInput")
    l_d = nc.dram_tensor("log_probs", [T, BL], bf16, kind="ExternalInput")
    e_d = nc.dram_tensor("entropies", [T, BL], bf16, kind="ExternalInput")
    m_d = nc.dram_tensor("to_include", [T, BL], bf16, kind="ExternalInput")
    acc_d = nc.dram_tensor("acc_out", [P, NACT * NCHUNK], f32, kind="ExternalOutput")
    pes_d = nc.dram_tensor("pe_stats", [NPE, BL], f32, kind="ExternalOutput")

    qi = np.arange(P)
    # scan lhsT[q, p] = gamma^(q-p) for q >= p (lower triangular)
    scan_np = np.tril(GAMMA ** (qi[:, None] - qi[None, :])).astype(np.float32)
    scan_d = nc.inline_tensor(scan_np, "scanmat")
    # carry selector lhsT[q, p] = gamma^(P-p) iff q == 0:
    # out[p, n] = gamma^(P-p) * R_next[0, n]
    sel_np = np.zeros((P, P), dtype=np.float32)
    sel_np[0, :] = GAMMA ** (P - qi)
    sel_d = nc.inline_tensor(sel_np, "selmat")
    # ones-column matrices for the PE stat reductions: oneh[:, j*NPE + k] = (k == j)
    import ml_dtypes
    oneh_np = np.zeros((P, NPE * NPE), dtype=np.float32)
    for j in range(NPE):
        oneh_np[:, j * NPE + j] = 1.0
    oneh_d = nc.inline_tensor(oneh_np.astype(ml_dtypes.bfloat16), "onehmat")

    with ExitStack() as ctx:
        def sb(name, shape, dtype):
            return ctx.enter_context(nc.sbuf_tensor(name, list(shape), dtype))

        scan_sb = sb("scan_sb", (P, P), f32r)
        sel_sb = sb("sel_sb", (P, P), f32r)
        oneh_sb = sb("oneh_sb", (P, NPE * NPE), bf16)
        r4 = [sb(f"r4_{i}", (P, KPC, BL), f32r) for i in range(2)]
        v4 = [sb(f"v4_{i}", (P, KPC, BL), bf16) for i in range(2)]
        l4 = [sb(f"l4_{i}", (P, KPC, BL), bf16) for i in range(2)]
        e4 = [sb(f"e4_{i}", (P, KPC, BL), bf16) for i in range(2)]
        m4 = [sb(f"m4_{i}", (P, KPC, BL), bf16) for i in range(2)]
        R_sb = [sb(f"R_sb_{i}", (P, BL), f32r) for i in range(3)]
        R_bf = [sb(f"R_bf_{i}", (P, KPC, BL), bf16) for i in range(2)]
        # DVE products, batched per super-chunk (double-buffered by super parity)
        mR = [sb(f"mR_{i}", (P, KPC, BL), bf16) for i in range(2)]
        mV = [sb(f"mV_{i}", (P, KPC, BL), bf16) for i in range(2)]
        mL = [sb(f"mL_{i}", (P, KPC, BL), bf16) for i in range(2)]
        pRR = [sb(f"pRR_{i}", (P, KPC, BL), bf16) for i in range(2)]
        pRV = [sb(f"pRV_{i}", (P, KPC, BL), bf16) for i in range(2)]
        pLR = [sb(f"pLR_{i}", (P, KPC, BL), bf16) for i in range(2)]
        pLV = [sb(f"pLV_{i}", (P, KPC, BL), bf16) for i in range(2)]
        # GPS products
        pME = [sb(f"pME_{i}", (P, KPC, BL), bf16) for i in range(2)]
        pVV = [sb(f"pVV_{i}", (P, KPC, BL), bf16) for i in range(2)]
        acc = sb("acc", (P, NACT * NCHUNK), f32)
        stats_sb = sb("stats_sb", (NPE, BL), f32)
        R_ps = [ctx.enter_context(nc.psum_tensor(f"R_ps_{i}", [P, BL], f32))
                for i in range(2)]
        st_ps = ctx.enter_context(nc.psum_tensor("st_ps", [NPE, BL], f32))

        def acol(stat, c):
            col = ACT_STATS.index(stat) * NCHUNK + c
            return acc[:, col:col + 1]

        def nsame(s):
            return (NSUPER - 1 - s) // 2 + 1

        with nc.Block() as block, \
                nc.semaphore("const_sem") as const_sem, \
                nc.semaphore("dma_even") as dma_even, \
                nc.semaphore("dma_odd") as dma_odd, \
                nc.semaphore("pe_scan") as pe_scan, \
                nc.semaphore("pe_done") as pe_done, \
                nc.semaphore("act_rc") as act_rc, \
                nc.semaphore("act_red") as act_red, \
                nc.semaphore("dve_l1") as dve_l1, \
                nc.semaphore("dve_l2") as dve_l2, \
                nc.semaphore("gps_done") as gps_done, \
                nc.semaphore("act_fin") as act_fin, \
                nc.semaphore("dma_out") as dma_out:
            dma_par = (dma_even, dma_odd)

            @block.sync
            def _(sync):
                sync.dma_start(out=scan_sb[:], in_=scan_d[:].bitcast(f32r)).then_inc(const_sem, 16)
                sync.dma_start(out=sel_sb[:], in_=sel_d[:].bitcast(f32r)).then_inc(const_sem, 16)
                sync.dma_start(out=oneh_sb[:], in_=oneh_d[:]).then_inc(const_sem, 16)
                for s in reversed(range(NSUPER)):
                    if s <= NSUPER - 3:
                        done = NCHUNK - KPC * (s + 2)
                        sync.wait_ge(pe_done, done)      # PE reduce groups (r4, m4)
                        sync.wait_ge(dve_l1, NSUPER - 2 - s)   # DVE level-1 (v4, l4, m4)
                        sync.wait_ge(gps_done, 2 * (NSUPER - 2 - s))  # GPS (m4, e4)
                    sl = s % 2
                    rows = slice(s * P * KPC, (s + 1) * P * KPC)
                    for dst, src in ((r4[sl], r_d), (v4[sl], v_d), (l4[sl], l_d),
                                     (e4[sl], e_d), (m4[sl], m_d)):
                        sync.dma_start(
                            out=dst[:],
                            in_=src[rows, :].rearrange("(k p) n -> p k n", p=P),
                        ).then_inc(dma_par[sl], 16)
                sync.wait_ge(act_red, NACT * NCHUNK)
                sync.wait_ge(act_fin, 1)
                sync.dma_start(out=acc_d[:], in_=acc[:]).then_inc(dma_out, 16)
                sync.dma_start(out=pes_d[:], in_=stats_sb[:]).then_inc(dma_out, 16)
                sync.wait_ge(dma_out, 32)

            def pe_reduces(pe, c):
                """stat-reduction matmuls for chunk c (emitted 4 iters later)"""
                s, k = divmod(c, KPC)
                sl = s % 2
                pe.wait_ge(dve_l2, NSUPER - s)
                pe.wait_ge(gps_done, 2 * (NSUPER - s))
                srcs = {
                    "N": m4[sl][:, k, :], "S1": mR[sl][:, k, :], "SV": mV[sl][:, k, :],
                    "SLP": mL[sl][:, k, :], "S2": pRR[sl][:, k, :], "SRV": pRV[sl][:, k, :],
                    "SLPR": pLR[sl][:, k, :], "SV2": pVV[sl][:, k, :],
                }
                start = c == NCHUNK - 1
                for j, stat in enumerate(PE_STATS):
                    mm = pe.matmul(st_ps[:], lhsT=oneh_sb[:, j * NPE:(j + 1) * NPE],
                                   rhs=srcs[stat],
                                   start=(start and j == 0),
                                   stop=(c == 0 and j == NPE - 1))
                    if stat == PE_STATS[-1]:
                        mm.then_inc(pe_done, 1)

            @block.tensor
            def _(pe):
                pe.wait_ge(const_sem, 48)
                for c in reversed(range(NCHUNK)):
                    s, k = divmod(c, KPC)
                    if k == KPC - 1:
                        pe.wait_ge(dma_par[s % 2], 80 * nsame(s))
                    if c <= NCHUNK - 3:
                        # R_ps bank c%2 must be fully drained by ACT (conv of c+2)
                        pe.wait_ge(act_rc, 2 * (NCHUNK - 2 - c))
                    rv = r4[s % 2][:, k, :]
                    ps = R_ps[c % 2]
                    if c == NCHUNK - 1:
                        mm = pe.matmul(ps[:], lhsT=scan_sb[:], rhs=rv,
                                       start=True, stop=True)
                    else:
                        pe.matmul(ps[:], lhsT=scan_sb[:], rhs=rv,
                                  start=True, stop=False)
                        # R_sb[c+1] written by ACT copy (odd act_rc increments)
                        pe.wait_ge(act_rc, 2 * (NCHUNK - 2 - c) + 1)
                        mm = pe.matmul(ps[:], lhsT=sel_sb[:],
                                       rhs=R_sb[(c + 1) % 3][:],
                                       start=False, stop=True)
                    mm.then_inc(pe_scan, 1)
                    if c < NCHUNK - 4:
                        pe_reduces(pe, c + 4)
                for c in (3, 2, 1, 0):
                    pe_reduces(pe, c)

            def act_reduces(act, c):
                # in-place copies: out == in avoids an unsynced scratch tile;
                # the accum_out is the real result
                s, k = divmod(c, KPC)
                sl = s % 2
                act.wait_ge(gps_done, 2 * (NSUPER - s) - 1)  # pME of super s
                act.activation(pME[sl][:, k, :], pME[sl][:, k, :], Copy,
                               accum_out=acol("SE", c)).then_inc(act_red, 1)
                act.wait_ge(dve_l2, NSUPER - s)
                act.activation(pLV[sl][:, k, :], pLV[sl][:, k, :], Copy,
                               accum_out=acol("SLPV", c)).then_inc(act_red, 1)

            @block.scalar
            def _(act):
                for c in reversed(range(NCHUNK)):
                    act.wait_ge(pe_scan, NCHUNK - c)
                    # R_sb slot WAR: rank1 of c+2 read slot (c+3)%3 == c%3
                    # covered by pe_scan wait above (rank1(c) done => rank1(c+2) done)
                    act.activation(R_sb[c % 3][:], R_ps[c % 2][:], Copy) \
                        .then_inc(act_rc, 1)
                    s, k = divmod(c, KPC)
                    if k == KPC - 1 and s <= NSUPER - 3:
                        # R_bf slot WAR: DVE level-1 of super s+2 read R_bf[s%2]
                        act.wait_ge(dve_l1, NSUPER - 2 - s)
                    act.activation(R_bf[s % 2][:, k, :], R_ps[c % 2][:], Copy) \
                        .then_inc(act_rc, 1)
                    if c < NCHUNK - 4:
                        act_reduces(act, c + 4)
                for cc in (3, 2, 1, 0):
                    act_reduces(act, cc)
                act.wait_ge(pe_done, NCHUNK)
                act.activation(stats_sb[:], st_ps[:], Copy).then_inc(act_fin, 1)

            @block.vector
            def _(dve):
                for s in reversed(range(NSUPER)):
                    sl = s % 2
                    dve.wait_ge(dma_par[sl], 80 * nsame(s))
                    # R_bf for all 4 chunks of super s (conv of chunk 4s last)
                    dve.wait_ge(act_rc, 2 * (NCHUNK - KPC * s))
                    if s <= NSUPER - 3:
                        # product tiles (sl) reused from super s+2: readers
                        dve.wait_ge(pe_done, NCHUNK - KPC * (s + 2))
                        dve.wait_ge(act_red, NACT * (NCHUNK - KPC * (s + 2)))
                        dve.wait_ge(gps_done, 2 * (NSUPER - 2 - s))
                    mv_in = m4[sl][:]
                    dve.tensor_tensor(out=mR[sl][:], in0=mv_in, in1=R_bf[sl][:], op=mult)
                    dve.tensor_tensor(out=mV[sl][:], in0=mv_in, in1=v4[sl][:], op=mult)
                    dve.tensor_tensor(out=mL[sl][:], in0=mv_in, in1=l4[sl][:], op=mult) \
                        .then_inc(dve_l1, 1)
                    # self-wait on dve_l1 orders level-2 after the level-1
                    # writes are committed (much cheaper than a full DRAIN)
                    dve.wait_ge(dve_l1, NSUPER - s)
                    dve.tensor_tensor(out=pRR[sl][:], in0=mR[sl][:], in1=mR[sl][:], op=mult)
                    dve.tensor_tensor(out=pRV[sl][:], in0=mR[sl][:], in1=mV[sl][:], op=mult)
                    dve.tensor_tensor(out=pLR[sl][:], in0=mL[sl][:], in1=mR[sl][:], op=mult)
                    dve.tensor_tensor(out=pLV[sl][:], in0=mL[sl][:], in1=mV[sl][:], op=mult) \
                        .then_inc(dve_l2, 1)

            @block.gpsimd
            def _(gps):
                for s in reversed(range(NSUPER)):
                    sl = s % 2
                    gps.wait_ge(dma_par[sl], 80 * nsame(s))
                    if s <= NSUPER - 3:
                        # pME/pVV tiles reused from super s+2: readers ACT, PE
                        gps.wait_ge(act_red, NACT * (NCHUNK - KPC * (s + 2)))
                        gps.wait_ge(pe_done, NCHUNK - KPC * (s + 2))
                    gps.tensor_tensor(out=pME[sl][:], in0=m4[sl][:],
                                      in1=e4[sl][:], op=mult).then_inc(gps_done, 1)
                    gps.wait_ge(dve_l1, NSUPER - s)
                    gps.tensor_tensor(out=pVV[sl][:], in0=mV[sl][:],
                                      in1=mV[sl][:], op=mult).then_inc(gps_done, 1)

    return nc


def _get_program():
    if "nc" not in _cache:
        _cache["nc"] = _build_program()
    return _cache["nc"]


def _shard_inputs(inputs):
    import ml_dtypes

    bf16 = ml_dtypes.bfloat16
    r = np.ascontiguousarray(inputs["rewards"], dtype=np.float32)
    v = np.asarray(inputs["value_estimates"], dtype=np.float32).astype(bf16)
    lp = np.asarray(inputs["log_probs"], dtype=np.float32).astype(bf16)
    e = np.asarray(inputs["entropies"], dtype=np.float32).astype(bf16)
    m = inputs["to_include"].astype(bf16)
    in_maps = []
    for c in range(NCORES):
        sl = slice(c * BL, (c + 1) * BL)
        in_maps.append({
            "rewards": np.ascontiguousarray(r[:, sl]),
            "value_estimates": np.ascontiguousarray(v[:, sl]),
            "log_probs": np.ascontiguousarray(lp[:, sl]),
            "entropies": np.ascontiguousarray(e[:, sl]),
            "to_include": np.ascontiguousarray(m[:, sl]),
        })
    return in_maps


def _execute(in_maps, trace=False):
    from concourse.bass_utils import run_bass_kernel_spmd

    nc = _get_program()
    res = run_bass_kernel_spmd(nc, in_maps, list(range(NCORES)), trace=trace)
    return res


def _stats_from_results(results):
    tot = {name: 0.0 for name in PE_STATS + ACT_STATS}
    for cm in results:
        pes = cm["pe_stats"].astype(np.float64)
        for j, name in enumerate(PE_STATS):
            tot[name] += pes[j].sum()
        ac = cm["acc_out"].astype(np.float64)
        for i, name in enumerate(ACT_STATS):
            tot[name] += ac[:, i * NCHUNK:(i + 1) * NCHUNK].sum()
    return tot


def _finalize(tot):
    N = tot["N"]; S1 = tot["S1"]; S2 = tot["S2"]
    SV = tot["SV"]; SRV = tot["SRV"]; SV2 = tot["SV2"]
    SLP = tot["SLP"]; SLPR = tot["SLPR"]; SLPV = tot["SLPV"]; SE = tot["SE"]
    mean = S1 / N
    q = S2 - 2.0 * mean * S1 + mean * mean * N   # sum(m*(R-mean)^2)
    var = q / (N - 1.0)
    s = np.sqrt(var) + EPS
    critic = q / (s * s) - 2.0 * (SRV - mean * SV) / s + SV2
    actor = -(SLPR - mean * SLP) / s + SLPV - ALPHA * SE
    return (np.float32(critic), np.float32(actor))


def kernel(**inputs):
    in_maps = _shard_inputs(inputs)
    res = _execute(in_maps, trace=False)
    tot = _stats_from_results(res.results)
    return _finalize(tot)
